# revision 7
# baseline (speedup 1.0000x reference)
import sys

if "/opt/trn_rl_repo" not in sys.path:
    sys.path.insert(0, "/opt/trn_rl_repo")

import numpy as np

from concourse import bacc, mybir, tile
from concourse.bass_utils import run_bass_kernel_spmd

N_CORES = 8
B, C, H, W = 4096, 2, 64, 64
BPC = B // N_CORES          # 512 batches per core
NS = BPC // 16              # 32 supertiles of 16 maps each
NG = 4                      # supertile groups of 8 (one DMA per group per plane)
NCHUNK = 8                  # data-loss chunks of [128, 4096] per tensor
CHUNK_F = 4096
GRID_D = 1.0 / (H - 1)
CLAMP_NEG_MIN = 27.6310211159  # -CLAMP_MIN

F32 = mybir.dt.float32
BF16 = mybir.dt.bfloat16
FP8 = mybir.dt.float8e4     # e4m3 is a PE perf dtype (streams at full rate)

# number of supertiles whose product pass is fed from a scalar-engine copy
# of the E-field (SBUF bf16) instead of reading PSUM f32 directly on DVE
N_OFFLOAD = 0


def _d1_unit(n):
    m = np.zeros((n, n), dtype=np.float64)
    for i in range(1, n - 1):
        m[i, i - 1], m[i, i + 1] = -0.5, 0.5
    m[0, 0:3] = [-1.5, 2.0, -0.5]
    m[-1, -1], m[-1, -2], m[-1, -3] = 1.5, -2.0, 0.5
    return m


def _d2_unit(n):
    m = np.zeros((n, n), dtype=np.float64)
    for i in range(1, n - 1):
        m[i, i - 1], m[i, i], m[i, i + 1] = 1.0, -2.0, 1.0
    m[0, 0:4] = [2.0, -5.0, 4.0, -1.0]
    m[-1, -1], m[-1, -2], m[-1, -3], m[-1, -4] = 2.0, -5.0, 4.0, -1.0
    return m


def _build_consts():
    import ml_dtypes

    bf = ml_dtypes.bfloat16
    f8 = ml_dtypes.float8_e4m3

    # unit-spacing operator: E = Ehat / d^2. Ehat entries are quarter-integers
    # in [-4.5, 8] — exact in e4m3 (verified numerically). The 1/d^2 = 3969 factor is applied on the
    # host at the end.
    d1 = _d1_unit(H)
    e_hat = -(_d2_unit(H) + d1.T @ d1)
    g1 = d1[H - 1, :] - d1[0, :]
    a = np.linalg.solve(e_hat.T, g1 * GRID_D)

    # lhsT for the E matmul: out = lhsT.T @ rhs must be blkdiag(Ehat,Ehat) @ rhs
    c_e = np.zeros((128, 128), dtype=f8)
    c_e[0:64, 0:64] = e_hat.T.astype(f8)
    c_e[64:128, 64:128] = e_hat.T.astype(f8)
    assert np.abs(c_e[0:64, 0:64].astype(np.float64) - e_hat.T).max() == 0.0

    # Banded reduction weights: slicing cols [63-2s : 127-2s] of this gives a
    # [128, 64] lhsT whose only nonzero columns are 2s (partitions 0:64) and
    # 2s+1 (partitions 64:128) — supertile s's partition-sums land in PSUM
    # rows 2s, 2s+1 while start=False accumulation leaves other rows alone.
    c_ones = np.zeros((128, 128), dtype=bf)
    for p in range(128):
        c_ones[p, 63 + p // 64] = 1.0

    return {"cE": c_e, "cOnes": c_ones}, a


def _build_nc():
    nc = bacc.Bacc("TRN2", target_bir_lowering=False, debug=False)

    # pb: interleaved (p | pT) supertile planes, kb: (K + a | KT - a)
    pb_d = nc.dram_tensor("pb", [NG, 128, 8192], FP8, kind="ExternalInput")
    kb_d = nc.dram_tensor("kb", [NG, 128, 8192], BF16, kind="ExternalInput")
    mo_d = nc.dram_tensor("mo", [NCHUNK, 128, CHUNK_F], BF16, kind="ExternalInput")
    tg_d = nc.dram_tensor("tg", [NCHUNK, 128, CHUNK_F], BF16, kind="ExternalInput")
    ce_d = nc.dram_tensor("cE", [128, 128], FP8, kind="ExternalInput")
    cones_d = nc.dram_tensor("cOnes", [128, 128], BF16, kind="ExternalInput")

    s1_out = nc.dram_tensor("s1", [64, 512], F32, kind="ExternalOutput")
    dstat_out = nc.dram_tensor("dstat", [128, NCHUNK], F32, kind="ExternalOutput")

    with tile.TileContext(nc) as tc:
        with (
            tc.tile_pool(name="consts", bufs=1) as cpool,
            tc.tile_pool(name="xin", bufs=2) as xpool,
            tc.tile_pool(name="din", bufs=2) as dpool,
            tc.tile_pool(name="uwork", bufs=4) as upool,
            tc.tile_pool(name="pep", bufs=3, space="PSUM") as ppool,
            tc.tile_pool(name="pacc", bufs=1, space="PSUM") as papool,
        ):
            ce = cpool.tile([128, 128], FP8, tag="ce")
            cones = cpool.tile([128, 128], BF16, tag="cones")
            dstat = cpool.tile([128, NCHUNK], F32, tag="dstat")
            nc.sync.dma_start(ce[:], ce_d[:])
            nc.sync.dma_start(cones[:], cones_d[:])

            sall = papool.tile([64, 512], F32, tag="sall")

            # software pipeline: the banded reduction for supertile s issues
            # two iterations later so the PE's in-order stream never stalls
            # on the DVE product + gpsimd fold of a recent supertile.
            from collections import deque

            pending = deque()  # (v_tile, s)

            def flush_banded(depth, last=False):
                while len(pending) > depth:
                    v_p, s_p = pending.popleft()
                    lo, hi = 63 - 2 * s_p, 127 - 2 * s_p
                    nc.tensor.matmul(
                        sall[:], cones[:, lo:hi], v_p[:],
                        start=(s_p == 0),
                        stop=(last and not pending),
                        skip_group_check=True,
                    )

            # data-loss sub is sliced so each DVE instruction stays short and
            # the residual-path products are never stuck behind a long sub.
            dsub = []  # (mt, tt, k, emitted_slices)

            def emit_sub_slice():
                if not dsub:
                    return
                mt, tt, k, done = dsub[0]
                c = 1024 * done
                nc.vector.tensor_tensor(
                    mt[:, c : c + 1024], mt[:, c : c + 1024], tt[:, c : c + 1024],
                    op=mybir.AluOpType.subtract,
                )
                if done == 3:
                    nc.scalar.activation(
                        mt[:], mt[:],
                        mybir.ActivationFunctionType.Square,
                        accum_out=dstat[:, k : k + 1],
                    )
                    dsub.pop(0)
                else:
                    dsub[0] = (mt, tt, k, done + 1)

            for g in range(NG):
                pb_t = xpool.tile([128, 8192], FP8, tag="pb")
                kb_t = xpool.tile([128, 8192], BF16, tag="kb")
                nc.sync.dma_start(pb_t[:], pb_d[g])
                # kb is 2MB bf16: split across both queues to balance bytes
                nc.scalar.dma_start(kb_t[:, 0:4096], kb_d[g, :, 0:4096])
                nc.scalar.dma_start(kb_t[:, 4096:8192], kb_d[g, :, 4096:8192])
                # prefetch this group's data-loss chunks right away so the
                # queues never idle behind compute on the trigger engines
                for k in (2 * g, 2 * g + 1):
                    mt = dpool.tile([128, CHUNK_F], BF16, tag="mt")
                    tt = dpool.tile([128, CHUNK_F], BF16, tag="tt")
                    if k % 2 == 0:
                        nc.sync.dma_start(mt[:], mo_d[k])
                        nc.scalar.dma_start(tt[:], tg_d[k])
                    else:
                        nc.scalar.dma_start(mt[:], mo_d[k])
                        nc.sync.dma_start(tt[:], tg_d[k])
                    dsub.append((mt, tt, k, 0))

                for t in range(8):
                    s = 8 * g + t
                    c0 = 1024 * t
                    ep = ppool.tile([128, 1024], F32, tag="ep")
                    nc.tensor.matmul(
                        ep[:, 0:512], ce[:], pb_t[:, c0 : c0 + 512],
                        start=True, stop=True,
                    )
                    nc.tensor.matmul(
                        ep[:, 512:1024], ce[:], pb_t[:, c0 + 512 : c0 + 1024],
                        start=True, stop=True,
                    )
                    flush_banded(2)
                    u = upool.tile([128, 1024], BF16, tag="u")
                    nc.vector.tensor_tensor(
                        u[:], kb_t[:, c0 : c0 + 1024], ep[:],
                        op=mybir.AluOpType.mult,
                    )
                    # fold the two directions so one banded matmul covers both
                    v = upool.tile([128, 512], BF16, tag="v")
                    nc.gpsimd.tensor_tensor(
                        v[:], u[:, 0:512], u[:, 512:1024],
                        op=mybir.AluOpType.add,
                    )
                    pending.append((v, s))
                    emit_sub_slice()

            while dsub:
                emit_sub_slice()
            flush_banded(0, last=True)

            s1_sb = cpool.tile([64, 512], F32, tag="s1sb")
            nc.scalar.copy(s1_sb[:], sall[:])
            nc.sync.dma_start(s1_out[:], s1_sb[:])
            nc.sync.dma_start(dstat_out[:], dstat[:])

    nc.compile()
    return nc


_NC = None
_CONSTS = None
_AVEC = None
LAST_RESULTS = None


def _arrange(x):
    # x: (g, t, r, j, h', w') -> (g, r, h', t, j*w') = (4, 2, 64, 8, 512)
    return np.ascontiguousarray(x.transpose(0, 2, 4, 1, 3, 5)).reshape(
        NG, 2, 64, 8, 512
    )


def kernel(model_out, target, x0_hat, var, _trace=False, _trace_kwargs=None):
    global _NC, _CONSTS, _AVEC, LAST_RESULTS
    if _NC is None:
        _CONSTS, _AVEC = _build_consts()
        _NC = _build_nc()

    import ml_dtypes

    bf = ml_dtypes.bfloat16
    f8 = ml_dtypes.float8_e4m3
    model_out = np.asarray(model_out).astype(bf)
    target = np.asarray(target).astype(bf)
    x0_hat = np.asarray(x0_hat, dtype=np.float32)
    var = np.asarray(var, dtype=np.float64)
    a = _AVEC

    in_maps = []
    for c in range(N_CORES):
        lo, hi = c * BPC, (c + 1) * BPC
        arr = x0_hat[lo:hi].reshape(NG, 8, 2, 8, 2, H, W)  # (g,t,r,j,ch,h,w)
        p = arr[:, :, :, :, 0]
        K = arr[:, :, :, :, 1]
        pn = _arrange(p)
        pt = _arrange(p.transpose(0, 1, 2, 3, 5, 4))
        kn = _arrange(K + a[None, None, None, None, :, None].astype(np.float32))
        kt = _arrange(
            K.transpose(0, 1, 2, 3, 5, 4)
            - a[None, None, None, None, :, None].astype(np.float32)
        )
        # interleave halves per supertile: free = (t, half, 512)
        pboth = np.stack([pn, pt], axis=4).reshape(NG, 128, 8192).astype(f8)
        kboth = np.stack([kn, kt], axis=4).reshape(NG, 128, 8192).astype(bf)
        in_maps.append(
            {
                "pb": pboth,
                "kb": kboth,
                "mo": model_out[lo:hi].reshape(NCHUNK, 128, CHUNK_F),
                "tg": target[lo:hi].reshape(NCHUNK, 128, CHUNK_F),
                **_CONSTS,
            }
        )

    kwargs = {}
    if _trace:
        kwargs["trace"] = True
        if _trace_kwargs:
            kwargs.update(_trace_kwargs)
    res = run_bass_kernel_spmd(_NC, in_maps, list(range(N_CORES)), **kwargs)
    LAST_RESULTS = res

    data_sum = 0.0
    nll_sum = 0.0
    scale = (H - 1) * (H - 1) / float(H * W * 3)
    for c in range(N_CORES):
        out = res.results[c]
        s1 = out["s1"].astype(np.float64)        # [64, 512]
        dstat = out["dstat"].astype(np.float64)  # [128, 8]

        # s1[2s+r, 64j+w] -> batch 16s + 8r + j
        r = s1.reshape(NS, 2, 8, 64).sum(axis=-1).reshape(BPC) * scale
        v = var[c * BPC : (c + 1) * BPC]
        nll = np.minimum(0.5 * r * r / v, CLAMP_NEG_MIN)
        nll_sum += nll.sum()
        data_sum += dstat.sum()

    loss = data_sum / (B * C * H * W) + nll_sum / B
    return np.float32(loss)


# revision 8
# speedup vs baseline: 1.1682x; 1.1682x over previous
import sys

if "/opt/trn_rl_repo" not in sys.path:
    sys.path.insert(0, "/opt/trn_rl_repo")

import numpy as np

from concourse import bacc, mybir, tile
from concourse.bass_utils import run_bass_kernel_spmd

N_CORES = 8
B, C, H, W = 4096, 2, 64, 64
BPC = B // N_CORES          # 512 batches per core
NS = BPC // 16              # 32 supertiles of 16 maps each
NG = 4                      # supertile groups of 8 (one DMA per group per plane)
NCHUNK = 8                  # data-loss chunks of [128, 4096] per tensor
CHUNK_F = 4096
GRID_D = 1.0 / (H - 1)
CLAMP_NEG_MIN = 27.6310211159  # -CLAMP_MIN

F32 = mybir.dt.float32
BF16 = mybir.dt.bfloat16
FP8 = mybir.dt.float8e4     # e4m3 is a PE perf dtype (streams at full rate)

# number of supertiles whose product pass is fed from a scalar-engine copy
# of the E-field (SBUF bf16) instead of reading PSUM f32 directly on DVE
N_OFFLOAD = 0


def _d1_unit(n):
    m = np.zeros((n, n), dtype=np.float64)
    for i in range(1, n - 1):
        m[i, i - 1], m[i, i + 1] = -0.5, 0.5
    m[0, 0:3] = [-1.5, 2.0, -0.5]
    m[-1, -1], m[-1, -2], m[-1, -3] = 1.5, -2.0, 0.5
    return m


def _d2_unit(n):
    m = np.zeros((n, n), dtype=np.float64)
    for i in range(1, n - 1):
        m[i, i - 1], m[i, i], m[i, i + 1] = 1.0, -2.0, 1.0
    m[0, 0:4] = [2.0, -5.0, 4.0, -1.0]
    m[-1, -1], m[-1, -2], m[-1, -3], m[-1, -4] = 2.0, -5.0, 4.0, -1.0
    return m


def _build_consts():
    import ml_dtypes

    bf = ml_dtypes.bfloat16
    f8 = ml_dtypes.float8_e4m3

    # unit-spacing operator: E = Ehat / d^2. Ehat entries are quarter-integers
    # in [-4.5, 8] — exact in e4m3 (verified numerically). The 1/d^2 = 3969 factor is applied on the
    # host at the end.
    d1 = _d1_unit(H)
    e_hat = -(_d2_unit(H) + d1.T @ d1)
    g1 = d1[H - 1, :] - d1[0, :]
    a = np.linalg.solve(e_hat.T, g1 * GRID_D)

    # lhsT for the E matmul: out = lhsT.T @ rhs must be blkdiag(Ehat,Ehat) @ rhs
    c_e = np.zeros((128, 128), dtype=f8)
    c_e[0:64, 0:64] = e_hat.T.astype(f8)
    c_e[64:128, 64:128] = e_hat.T.astype(f8)
    assert np.abs(c_e[0:64, 0:64].astype(np.float64) - e_hat.T).max() == 0.0

    # Banded reduction weights: slicing cols [63-2s : 127-2s] of this gives a
    # [128, 64] lhsT whose only nonzero columns are 2s (partitions 0:64) and
    # 2s+1 (partitions 64:128) — supertile s's partition-sums land in PSUM
    # rows 2s, 2s+1 while start=False accumulation leaves other rows alone.
    c_ones = np.zeros((128, 128), dtype=bf)
    for p in range(128):
        c_ones[p, 63 + p // 64] = 1.0

    return {"cE": c_e, "cOnes": c_ones}, a


def _build_nc():
    nc = bacc.Bacc("TRN2", target_bir_lowering=False, debug=False)

    # pb: interleaved (p | pT) supertile planes, kb: (K + a | KT - a)
    pb_d = nc.dram_tensor("pb", [NG, 128, 8192], FP8, kind="ExternalInput")
    kb_d = nc.dram_tensor("kb", [NG, 128, 8192], BF16, kind="ExternalInput")
    mo_d = nc.dram_tensor("mo", [NCHUNK, 128, CHUNK_F], BF16, kind="ExternalInput")
    tg_d = nc.dram_tensor("tg", [NCHUNK, 128, CHUNK_F], BF16, kind="ExternalInput")
    ce_d = nc.dram_tensor("cE", [128, 128], FP8, kind="ExternalInput")
    cones_d = nc.dram_tensor("cOnes", [128, 128], BF16, kind="ExternalInput")

    s1_out = nc.dram_tensor("s1", [64, 512], F32, kind="ExternalOutput")
    dstat_out = nc.dram_tensor("dstat", [128, NCHUNK], F32, kind="ExternalOutput")

    with tile.TileContext(nc) as tc:
        with (
            tc.tile_pool(name="consts", bufs=1) as cpool,
            tc.tile_pool(name="xin", bufs=2) as xpool,
            tc.tile_pool(name="din", bufs=3) as dpool,
            tc.tile_pool(name="uwork", bufs=5) as upool,
            tc.tile_pool(name="pep", bufs=3, space="PSUM") as ppool,
            tc.tile_pool(name="pacc", bufs=1, space="PSUM") as papool,
        ):
            ce = cpool.tile([128, 128], FP8, tag="ce")
            cones = cpool.tile([128, 128], BF16, tag="cones")
            dstat = cpool.tile([128, NCHUNK], F32, tag="dstat")
            nc.sync.dma_start(ce[:], ce_d[:])
            nc.sync.dma_start(cones[:], cones_d[:])

            sall = papool.tile([64, 512], F32, tag="sall")

            # software pipeline: the banded reduction for supertile s issues
            # two iterations later so the PE's in-order stream never stalls
            # on the DVE product + gpsimd fold of a recent supertile.
            from collections import deque

            pending = deque()  # (v_tile, s)

            def flush_banded(depth, last=False):
                while len(pending) > depth:
                    v_p, s_p = pending.popleft()
                    lo, hi = 63 - 2 * s_p, 127 - 2 * s_p
                    nc.tensor.matmul(
                        sall[:], cones[:, lo:hi], v_p[:],
                        start=(s_p == 0),
                        stop=(last and not pending),
                        skip_group_check=True,
                    )

            # data-loss sub is sliced so each DVE instruction stays short and
            # the residual-path products are never stuck behind a long sub.
            dsub = []  # (mt, tt, k, emitted_slices)

            def emit_sub_slice():
                if not dsub:
                    return
                mt, tt, k, done = dsub[0]
                c = 1024 * done
                nc.vector.tensor_tensor(
                    mt[:, c : c + 1024], mt[:, c : c + 1024], tt[:, c : c + 1024],
                    op=mybir.AluOpType.subtract,
                )
                if done == 3:
                    nc.scalar.activation(
                        mt[:], mt[:],
                        mybir.ActivationFunctionType.Square,
                        accum_out=dstat[:, k : k + 1],
                    )
                    dsub.pop(0)
                else:
                    dsub[0] = (mt, tt, k, done + 1)

            for g in range(NG):
                pb_t = xpool.tile([128, 8192], FP8, tag="pb")
                kb_t = xpool.tile([128, 8192], BF16, tag="kb")
                # 7 x 1MB transfers per group, interleaved across the two
                # HW queues (sync / scalar) so both stay ~equally loaded
                qa = nc.sync if g % 2 == 0 else nc.scalar
                qb = nc.scalar if g % 2 == 0 else nc.sync
                qa.dma_start(pb_t[:], pb_d[g])
                qb.dma_start(kb_t[:, 0:4096], kb_d[g, :, 0:4096])
                qa.dma_start(kb_t[:, 4096:8192], kb_d[g, :, 4096:8192])
                # prefetch this group's data-loss chunks right away so the
                # queues never idle behind compute on the trigger engines
                for n, k in enumerate((2 * g, 2 * g + 1)):
                    mt = dpool.tile([128, CHUNK_F], BF16, tag="mt")
                    tt = dpool.tile([128, CHUNK_F], BF16, tag="tt")
                    if n == 0:
                        qb.dma_start(mt[:], mo_d[k])
                        qa.dma_start(tt[:], tg_d[k])
                    else:
                        qa.dma_start(mt[:], mo_d[k])
                        qb.dma_start(tt[:], tg_d[k])
                    dsub.append((mt, tt, k, 0))

                for t in range(8):
                    s = 8 * g + t
                    c0 = 1024 * t
                    ep = ppool.tile([128, 1024], F32, tag="ep")
                    nc.tensor.matmul(
                        ep[:, 0:512], ce[:], pb_t[:, c0 : c0 + 512],
                        start=True, stop=True,
                    )
                    nc.tensor.matmul(
                        ep[:, 512:1024], ce[:], pb_t[:, c0 + 512 : c0 + 1024],
                        start=True, stop=True,
                    )
                    flush_banded(2)
                    u = upool.tile([128, 1024], BF16, tag="u")
                    nc.vector.tensor_tensor(
                        u[:], kb_t[:, c0 : c0 + 1024], ep[:],
                        op=mybir.AluOpType.mult,
                    )
                    # fold the two directions so one banded matmul covers both
                    v = upool.tile([128, 512], BF16, tag="v")
                    nc.gpsimd.tensor_tensor(
                        v[:], u[:, 0:512], u[:, 512:1024],
                        op=mybir.AluOpType.add,
                    )
                    pending.append((v, s))
                    emit_sub_slice()

            while dsub:
                emit_sub_slice()
            flush_banded(0, last=True)

            s1_sb = cpool.tile([64, 512], F32, tag="s1sb")
            nc.scalar.copy(s1_sb[:], sall[:])
            nc.sync.dma_start(s1_out[:], s1_sb[:])
            nc.sync.dma_start(dstat_out[:], dstat[:])

    nc.compile()
    return nc


_NC = None
_CONSTS = None
_AVEC = None
LAST_RESULTS = None


def _arrange(x):
    # x: (g, t, r, j, h', w') -> (g, r, h', t, j*w') = (4, 2, 64, 8, 512)
    return np.ascontiguousarray(x.transpose(0, 2, 4, 1, 3, 5)).reshape(
        NG, 2, 64, 8, 512
    )


def kernel(model_out, target, x0_hat, var, _trace=False, _trace_kwargs=None):
    global _NC, _CONSTS, _AVEC, LAST_RESULTS
    if _NC is None:
        _CONSTS, _AVEC = _build_consts()
        _NC = _build_nc()

    import ml_dtypes

    bf = ml_dtypes.bfloat16
    f8 = ml_dtypes.float8_e4m3
    model_out = np.asarray(model_out).astype(bf)
    target = np.asarray(target).astype(bf)
    x0_hat = np.asarray(x0_hat, dtype=np.float32)
    var = np.asarray(var, dtype=np.float64)
    a = _AVEC

    in_maps = []
    for c in range(N_CORES):
        lo, hi = c * BPC, (c + 1) * BPC
        arr = x0_hat[lo:hi].reshape(NG, 8, 2, 8, 2, H, W)  # (g,t,r,j,ch,h,w)
        p = arr[:, :, :, :, 0]
        K = arr[:, :, :, :, 1]
        pn = _arrange(p)
        pt = _arrange(p.transpose(0, 1, 2, 3, 5, 4))
        kn = _arrange(K + a[None, None, None, None, :, None].astype(np.float32))
        kt = _arrange(
            K.transpose(0, 1, 2, 3, 5, 4)
            - a[None, None, None, None, :, None].astype(np.float32)
        )
        # interleave halves per supertile: free = (t, half, 512)
        pboth = np.stack([pn, pt], axis=4).reshape(NG, 128, 8192).astype(f8)
        kboth = np.stack([kn, kt], axis=4).reshape(NG, 128, 8192).astype(bf)
        in_maps.append(
            {
                "pb": pboth,
                "kb": kboth,
                "mo": model_out[lo:hi].reshape(NCHUNK, 128, CHUNK_F),
                "tg": target[lo:hi].reshape(NCHUNK, 128, CHUNK_F),
                **_CONSTS,
            }
        )

    kwargs = {}
    if _trace:
        kwargs["trace"] = True
        if _trace_kwargs:
            kwargs.update(_trace_kwargs)
    res = run_bass_kernel_spmd(_NC, in_maps, list(range(N_CORES)), **kwargs)
    LAST_RESULTS = res

    data_sum = 0.0
    nll_sum = 0.0
    scale = (H - 1) * (H - 1) / float(H * W * 3)
    for c in range(N_CORES):
        out = res.results[c]
        s1 = out["s1"].astype(np.float64)        # [64, 512]
        dstat = out["dstat"].astype(np.float64)  # [128, 8]

        # s1[2s+r, 64j+w] -> batch 16s + 8r + j
        r = s1.reshape(NS, 2, 8, 64).sum(axis=-1).reshape(BPC) * scale
        v = var[c * BPC : (c + 1) * BPC]
        nll = np.minimum(0.5 * r * r / v, CLAMP_NEG_MIN)
        nll_sum += nll.sum()
        data_sum += dstat.sum()

    loss = data_sum / (B * C * H * W) + nll_sum / B
    return np.float32(loss)
